# revision 7
# baseline (speedup 1.0000x reference)
"""HardMiningLoss TRN2 kernel: n=8192, d=512, 8 NeuronCores, data-parallel rows.

Encoding: smneg[i,j] = 4*same(i,j) - sim(i,j).
  negatives (diff class): smneg = -sim   in [-1, 1]
  positives (same class): smneg = 4-sim  in [ 3, 5]
Mining reductions become threshold ops on smneg:
  min_pos = 4 - rowmax(smneg);  max_neg = -rowmin(smneg)
  neg_keep: smneg < alpha, alpha = rowmax - 3.9
  pos_keep: smneg > beta,  beta  = rowmin + 3.9

Host preprocessing sorts rows by class (original last row pinned to sorted
position n-1), so each 128-row chunk's same-class columns all fall inside a
256-col window [c*128, c*128+256) after a per-core column rotation of
(core*1024 - 64). Positive-side stats (rowmax, pos cnt/sum) are window ops.

Engine split per chunk (128 rows x 8192 cols):
  PE   : fp8e4 DoubleRow matmuls (2 k-pair passes of 256-contraction)
  DVE  : same4 mask, TTR evac of quarters 0,1 (fused row-min accum),
         window rowmax/pos ops, is_lt/min threshold scans on cols [0:SPL]
  ACT  : Copy(scale=-1) evac of quarters 2,3, Sign/Relu scans on [SPL:8192]
  Pool : min-reduce over quarters 2,3
Host finisher assembles the scalar loss from per-row linear accounting.
"""
import numpy as np
import ml_dtypes
from contextlib import ExitStack

import concourse.bass as bass
import concourse.tile as tile
from concourse import bacc, mybir
from concourse.bass_utils import run_bass_kernel_spmd

F32 = mybir.dt.float32
F16 = mybir.dt.float16
F8 = mybir.dt.float8e4
BF16 = mybir.dt.bfloat16
Alu = mybir.AluOpType
Act = mybir.ActivationFunctionType
AX = mybir.AxisListType.X
DR = mybir.MatmulPerfMode.DoubleRow

N_TOT, D, N_CORES = 8192, 512, 8
ROWS = N_TOT // N_CORES          # 1024 rows per core
CHUNKS = ROWS // 128             # 8 chunks of 128 rows
QCOLS = 2048                     # quarter width (half of PSUM x2 bufs)
NQ = N_TOT // QCOLS
KP = D // 256                    # 2 DoubleRow k-pair passes
PAD = 64                         # rotation pad so class windows start at col c*128
WIN = 256                        # window width covering all same-class cols
SPL = 6400                       # DVE scans [0:SPL], ACT scans [SPL:N_TOT]
RST = N_TOT - SPL
MARGIN = 0.1
INCLUDE_SELF_LAST_ROW = True

# stage column layout
C_MAX, C_MIN, C_G1, C_E1, C_S2, C_R2, C_PC, C_F = (
    0, 8, 16, 24, 32, 40, 48, 56)
C_PCALL, C_PSALL, C_SELF, C_QS = 64, 65, 66, 67
STAGE_W = 72


def build_program():
    nc = bacc.Bacc("TRN2", target_bir_lowering=False, debug=False)
    xt_d = nc.dram_tensor("xt", [128, KP * 2, N_TOT], F8, kind="ExternalInput")
    tb_d = nc.dram_tensor("tb", [128, QCOLS], F16, kind="ExternalInput")
    tp_d = nc.dram_tensor("tp", [128, CHUNKS], F32, kind="ExternalInput")
    st_d = nc.dram_tensor("stage", [128, STAGE_W], F32, kind="ExternalOutput")

    with tile.TileContext(nc) as tc, ExitStack() as ctx:
        pool = ctx.enter_context(tc.tile_pool(name="p", bufs=1))
        dbuf = ctx.enter_context(tc.tile_pool(name="db", bufs=2))
        pspool = ctx.enter_context(
            tc.tile_pool(name="ps", bufs=2, space=bass.MemorySpace.PSUM))

        xtb = pool.tile([128, KP * 2, N_TOT], F8)
        tb = pool.tile([128, QCOLS], F16)
        tp = pool.tile([128, CHUNKS], F32)
        stage = pool.tile([128, STAGE_W], F32)
        junk_d = pool.tile([128, SPL], F16)    # DVE scan outputs (ignored)
        junk_a = pool.tile([128, N_TOT], F8)       # ACT scan outputs (ignored)
        junk_w = pool.tile([128, WIN], F32)      # window outputs (f32: exact
                                                 # beta fill values in accum)

        nc.sync.dma_start(tb[:], tb_d.ap())
        nc.sync.dma_start(tp[:], tp_d.ap())
        for q in range(NQ):
            eng = nc.sync if q % 2 == 0 else nc.scalar
            eng.dma_start(xtb[:, :, q * QCOLS:(q + 1) * QCOLS],
                          xt_d.ap()[:, :, q * QCOLS:(q + 1) * QCOLS])

        for c in range(CHUNKS):
            same4 = dbuf.tile([128, QCOLS], F16, name="same4")
            smneg = dbuf.tile([128, N_TOT], F16, name="smneg")
            q0raw = dbuf.tile([128, QCOLS], F16, name="q0raw")
            alpha = dbuf.tile([128, 1], F32, name="alpha")
            alphan = dbuf.tile([128, 1], F32, name="alphan")
            beta = dbuf.tile([128, 1], F32, name="beta")
            w0 = c * 128
            # same4 = (tb == tp[:, c]) * 4    (only q0 cols can be same-class)
            nc.vector.tensor_scalar(same4[:], tb[:], tp[:, c:c + 1], 4.0,
                                    Alu.is_equal, Alu.mult)
            for q in range(NQ):
                ps = pspool.tile([128, QCOLS], F32)
                for nb in range(QCOLS // 512):
                    col = q * QCOLS + nb * 512
                    for p in range(KP):
                        nc.tensor.matmul(
                            ps[:, nb * 512:(nb + 1) * 512],
                            xtb[:, 2 * p:2 * p + 2,
                                PAD + c * 128:PAD + (c + 1) * 128],
                            xtb[:, 2 * p:2 * p + 2, col:col + 512],
                            start=(p == 0), stop=(p == KP - 1),
                            perf_mode=DR)
                # ACT evac: -sim (q0 to a staging tile, same4 merged below)
                acc = ({"accum_out": stage[:, C_QS + q:C_QS + q + 1]}
                       if c == CHUNKS - 1 else {})
                nc.scalar.activation(
                    q0raw[:] if q == 0 else smneg[:, q * QCOLS:(q + 1) * QCOLS],
                    ps[:], Act.Copy, bias=0.0, scale=-1.0, **acc)
            # merge class mask into quarter 0 (f16 2x TT)
            nc.vector.tensor_tensor(smneg[:, 0:QCOLS], same4[:], q0raw[:],
                                    Alu.add)
            # pairwise-min tree (DVE f16 2x) for the full-row min
            t = dbuf.tile([128, 6144], F16, name="tmin")
            nc.vector.tensor_tensor(t[:, 0:4096], smneg[:, 0:4096],
                                    smneg[:, 4096:8192], Alu.min)
            nc.vector.tensor_tensor(t[:, 4096:6144], t[:, 0:2048],
                                    t[:, 2048:4096], Alu.min)
            nc.vector.tensor_tensor(t[:, 0:1024], t[:, 4096:5120],
                                    t[:, 5120:6144], Alu.min)
            nc.vector.tensor_tensor(t[:, 1024:1536], t[:, 0:512],
                                    t[:, 512:1024], Alu.min)
            nc.vector.tensor_tensor(t[:, 0:256], t[:, 1024:1280],
                                    t[:, 1280:1536], Alu.min)
            nc.vector.tensor_reduce(stage[:, C_MIN + c:C_MIN + c + 1],
                                    t[:, 0:256], AX, Alu.min)
            # window rowmax -> stage
            nc.vector.tensor_reduce(stage[:, C_MAX + c:C_MAX + c + 1],
                                    smneg[:, w0:w0 + WIN], AX, Alu.max)
            # alpha = rowmax - 3.9 ; alphan = -alpha ; beta = rowmin + 3.9
            nc.vector.tensor_scalar(alpha[:], stage[:, C_MAX + c:C_MAX + c + 1],
                                    -3.9, None, Alu.add)
            nc.vector.tensor_scalar(alphan[:],
                                    stage[:, C_MAX + c:C_MAX + c + 1],
                                    -1.0, 3.9, Alu.mult, Alu.add)
            nc.vector.tensor_scalar(beta[:], stage[:, C_MIN + c:C_MIN + c + 1],
                                    3.9, None, Alu.add)
            # neg side scans: DVE on [0:SPL], ACT on [SPL:]
            nc.vector.tensor_scalar(junk_d[:, 0:SPL], smneg[:, 0:SPL],
                                    alpha[:], 0.0, Alu.is_lt, Alu.add,
                                    accum_out=stage[:, C_G1 + c:C_G1 + c + 1])
            nc.vector.tensor_scalar(junk_d[:, 0:SPL], smneg[:, 0:SPL],
                                    alpha[:], 0.0, Alu.min, Alu.add,
                                    accum_out=stage[:, C_E1 + c:C_E1 + c + 1])
            nc.scalar.activation(junk_a[:, 0:RST], smneg[:, SPL:], Act.Sign,
                                 bias=alphan[:], scale=1.0,
                                 accum_out=stage[:, C_S2 + c:C_S2 + c + 1])
            nc.scalar.activation(junk_a[:, 0:RST], smneg[:, SPL:], Act.Relu,
                                 bias=alpha[:], scale=-1.0,
                                 accum_out=stage[:, C_R2 + c:C_R2 + c + 1])
            # pos side: window ops
            nc.vector.tensor_scalar(junk_w[:], smneg[:, w0:w0 + WIN],
                                    beta[:], 0.0, Alu.is_gt, Alu.add,
                                    accum_out=stage[:, C_PC + c:C_PC + c + 1])
            nc.vector.tensor_scalar(junk_w[:], smneg[:, w0:w0 + WIN],
                                    beta[:], 0.0, Alu.max, Alu.add,
                                    accum_out=stage[:, C_F + c:C_F + c + 1])

            if c == CHUNKS - 1:
                # unmined last-row stats (row n-1 = partition 127, core 7)
                nc.vector.tensor_scalar(junk_w[:], smneg[:, w0:w0 + WIN],
                                        3.0, 0.0, Alu.is_gt, Alu.add,
                                        accum_out=stage[:, C_PCALL:C_PCALL + 1])
                nc.vector.tensor_scalar(junk_w[:], smneg[:, w0:w0 + WIN],
                                        3.0, 0.0, Alu.max, Alu.add,
                                        accum_out=stage[:, C_PSALL:C_PSALL + 1])
                selfc = PAD + c * 128 + 127
                nc.vector.tensor_copy(stage[:, C_SELF:C_SELF + 1],
                                      smneg[:, selfc:selfc + 1])

        nc.sync.dma_start(st_d.ap(), stage[:])
    nc.compile()
    return nc


_NC_CACHE = None


def kernel(inputs, targets, _want_time=False, _trace=False):
    global _NC_CACHE
    x = np.asarray(inputs, dtype=np.float32)
    tgt = np.asarray(targets).astype(np.int64)
    n = N_TOT

    # class-sort rows; pin original last row to sorted position n-1 so the
    # last-row stats land at core 7 / chunk 7 / partition 127
    c_star = tgt[n - 1]
    order = np.argsort(np.where(tgt == c_star, 1 << 20, tgt), kind="stable")
    xs = x[order]
    ts_ = tgt[order].astype(np.float32)
    x8 = xs.astype(ml_dtypes.float8_e4m3fn)

    if _NC_CACHE is None:
        _NC_CACHE = build_program()
    nc = _NC_CACHE

    in_maps = []
    for m in range(N_CORES):
        shift = (m * ROWS - PAD) % n
        cols = (np.arange(n) + shift) % n
        xr = x8[cols]                       # [n, d] rotated
        xt_m = np.ascontiguousarray(
            xr.T.reshape(KP * 2, 128, n).transpose(1, 0, 2))
        tb_m = np.ascontiguousarray(np.broadcast_to(
            ts_[cols[:QCOLS]][None, :], (128, QCOLS))).astype(np.float16)
        tp_m = np.ascontiguousarray(
            ts_[m * ROWS:(m + 1) * ROWS].reshape(CHUNKS, 128).T
        ).astype(np.float32)
        in_maps.append({"xt": xt_m, "tb": tb_m, "tp": tp_m})

    res = run_bass_kernel_spmd(nc, in_maps, core_ids=list(range(N_CORES)),
                               trace=_trace)

    # ---- host finisher ----
    maxS = np.empty(n); minS = np.empty(n)
    g1 = np.empty(n); e1 = np.empty(n)
    s2 = np.empty(n); r2 = np.empty(n)
    pcnt = np.empty(n); fsum = np.empty(n)
    last = None
    for m in range(N_CORES):
        st = np.asarray(res.results[m]["stage"], dtype=np.float64)
        for c in range(CHUNKS):
            rows = slice(m * ROWS + c * 128, m * ROWS + (c + 1) * 128)
            maxS[rows] = st[:, C_MAX + c]
            minS[rows] = st[:, C_MIN + c]
            g1[rows] = st[:, C_G1 + c]
            e1[rows] = st[:, C_E1 + c]
            s2[rows] = st[:, C_S2 + c]
            r2[rows] = st[:, C_R2 + c]
            pcnt[rows] = st[:, C_PC + c]
            fsum[rows] = st[:, C_F + c]
        if m == N_CORES - 1:
            last = st

    alpha = maxS - (4.0 - MARGIN)
    beta = minS + (4.0 - MARGIN)
    g1 = np.round(g1)
    ncnt2 = np.round((RST - s2) / 2.0)
    ncnt = g1 + ncnt2
    pcnt = np.round(pcnt)
    # sum of kept smneg: DVE half via min-accum, ACT half via relu-accum
    neg_sum_smneg = (e1 - alpha * (SPL - g1)) + (alpha * ncnt2 - r2)
    neg_sum_sim = -neg_sum_smneg
    pos_sum_smneg = fsum - beta * (WIN - pcnt)
    pos_sum_sim = 4.0 * pcnt - pos_sum_smneg

    pos_loss = (pcnt - pos_sum_sim) / np.maximum(pcnt, 1.0)
    neg_loss = neg_sum_sim / np.maximum(ncnt, 1.0)
    valid = ncnt >= 1.0
    loss = np.sum(np.where(valid, pos_loss + neg_loss, 0.0)) / n
    prec = np.sum(~valid) / n

    # last-row unmined stats (partition 127 of core 7 stage)
    pc_all = float(np.round(last[127, C_PCALL]))
    sum_smneg_pos = float(last[127, C_PSALL]) - 3.0 * (WIN - pc_all)
    selfv = float(last[127, C_SELF])
    # unmined neg side from full-row quarter sums + exact class size
    cls_size = float(np.sum(tgt == c_star))
    nc_all = n - cls_size
    t_raw = float(last[127, C_QS:C_QS + 4].sum())   # sum of -sim, full row
    sum_same_smneg = (float(last[127, C_PSALL]) - 3.0 * (WIN - pc_all)
                      + (selfv if selfv <= 3.0 else 0.0))
    w_same = 4.0 * cls_size - sum_same_smneg
    neg_sim_sum = -(t_raw + w_same)
    dev_included = selfv > 3.0            # device's sim_self < 1 decision
    if INCLUDE_SELF_LAST_ROW and not dev_included:
        pc_all += 1.0; sum_smneg_pos += selfv
    elif (not INCLUDE_SELF_LAST_ROW) and dev_included:
        pc_all -= 1.0; sum_smneg_pos -= selfv
    pos_sim_sum = 4.0 * pc_all - sum_smneg_pos
    mean_pos_sim = pos_sim_sum / max(pc_all, 1.0)
    mean_neg_sim = neg_sim_sum / max(nc_all, 1.0)

    out = np.array([loss, prec, mean_pos_sim, mean_neg_sim], dtype=np.float32)
    if _want_time:
        return out, res
    return out


# revision 9
# speedup vs baseline: 1.0336x; 1.0336x over previous
"""HardMiningLoss TRN2 kernel: n=8192, d=512, 8 NeuronCores, data-parallel rows.

Encoding: smneg[i,j] = 4*same(i,j) - sim(i,j).
  negatives (diff class): smneg = -sim   in [-1, 1]
  positives (same class): smneg = 4-sim  in [ 3, 5]
Mining reductions become threshold ops on smneg:
  min_pos = 4 - rowmax(smneg);  max_neg = -rowmin(smneg)
  neg_keep: smneg < alpha, alpha = rowmax - 3.9
  pos_keep: smneg > beta,  beta  = rowmin + 3.9

Host preprocessing sorts rows by class (original last row pinned to sorted
position n-1), so each 128-row chunk's same-class columns all fall inside a
256-col window [c*128, c*128+256) after a per-core column rotation of
(core*1024 - 64). Positive-side stats (rowmax, pos cnt/sum) are window ops.

Engine split per chunk (128 rows x 8192 cols):
  PE   : fp8e4 DoubleRow matmuls (2 k-pair passes of 256-contraction)
  DVE  : same4 mask, TTR evac of quarters 0,1 (fused row-min accum),
         window rowmax/pos ops, is_lt/min threshold scans on cols [0:SPL]
  ACT  : Copy(scale=-1) evac of quarters 2,3, Sign/Relu scans on [SPL:8192]
  Pool : min-reduce over quarters 2,3
Host finisher assembles the scalar loss from per-row linear accounting.
"""
import numpy as np
import ml_dtypes
from contextlib import ExitStack

import concourse.bass as bass
import concourse.tile as tile
from concourse import bacc, mybir
from concourse.bass_utils import run_bass_kernel_spmd

F32 = mybir.dt.float32
F16 = mybir.dt.float16
F8 = mybir.dt.float8e4
BF16 = mybir.dt.bfloat16
Alu = mybir.AluOpType
Act = mybir.ActivationFunctionType
AX = mybir.AxisListType.X
DR = mybir.MatmulPerfMode.DoubleRow

N_TOT, D, N_CORES = 8192, 512, 8
ROWS = N_TOT // N_CORES          # 1024 rows per core
CHUNKS = ROWS // 128             # 8 chunks of 128 rows
QCOLS = 2048                     # quarter width (half of PSUM x2 bufs)
NQ = N_TOT // QCOLS
KP = D // 256                    # 2 DoubleRow k-pair passes
PAD = 64                         # rotation pad so class windows start at col c*128
WIN = 256                        # window width covering all same-class cols
SPL = 6400                       # DVE scans [0:SPL], ACT scans [SPL:N_TOT]
RST = N_TOT - SPL
MARGIN = 0.1
INCLUDE_SELF_LAST_ROW = True

# stage column layout
C_MAX, C_MIN, C_G1, C_E1, C_S2, C_R2, C_PC, C_F = (
    0, 8, 16, 24, 32, 40, 48, 56)
C_PCALL, C_PSALL, C_SELF, C_QS = 64, 65, 66, 67
STAGE_W = 72


def build_program():
    nc = bacc.Bacc("TRN2", target_bir_lowering=False, debug=False)
    xt_d = nc.dram_tensor("xt", [128, KP * 2, N_TOT], F8, kind="ExternalInput")
    tb_d = nc.dram_tensor("tb", [128, QCOLS], F16, kind="ExternalInput")
    tp_d = nc.dram_tensor("tp", [128, CHUNKS], F32, kind="ExternalInput")
    st_d = nc.dram_tensor("stage", [128, STAGE_W], F32, kind="ExternalOutput")

    with tile.TileContext(nc) as tc, ExitStack() as ctx:
        pool = ctx.enter_context(tc.tile_pool(name="p", bufs=1))
        dbuf = ctx.enter_context(tc.tile_pool(name="db", bufs=2))
        pspool = ctx.enter_context(
            tc.tile_pool(name="ps", bufs=2, space=bass.MemorySpace.PSUM))

        xtb = [pool.tile([128, KP * 2, QCOLS], F8, name=f"xtb{q}")
               for q in range(NQ)]
        tb = pool.tile([128, QCOLS], F16)
        tp = pool.tile([128, CHUNKS], F32)
        stage = pool.tile([128, STAGE_W], F32)
        junk_d = pool.tile([128, SPL], F16)    # DVE scan outputs (ignored)
        junk_a = pool.tile([128, N_TOT], F8)       # ACT scan outputs (ignored)
        junk_w = pool.tile([128, WIN], F32)      # window outputs (f32: exact
                                                 # beta fill values in accum)

        # q0 first: it carries the weights for every chunk, so compute can
        # start as soon as it (plus tb/tp) lands
        nc.sync.dma_start(xtb[0][:], xt_d.ap()[:, :, 0:QCOLS])
        nc.sync.dma_start(tb[:], tb_d.ap())
        nc.sync.dma_start(tp[:], tp_d.ap())
        for q in range(1, NQ):
            eng = nc.sync if q % 2 == 0 else nc.scalar
            eng.dma_start(xtb[q][:], xt_d.ap()[:, :, q * QCOLS:(q + 1) * QCOLS])

        for c in range(CHUNKS):
            same4 = dbuf.tile([128, QCOLS], F16, name="same4")
            smneg = dbuf.tile([128, N_TOT], F16, name="smneg")
            q0raw = dbuf.tile([128, QCOLS], F16, name="q0raw")
            alpha = dbuf.tile([128, 1], F32, name="alpha")
            alphan = dbuf.tile([128, 1], F32, name="alphan")
            beta = dbuf.tile([128, 1], F32, name="beta")
            w0 = c * 128
            # same4 = (tb == tp[:, c]) * 4    (only q0 cols can be same-class)
            nc.vector.tensor_scalar(same4[:], tb[:], tp[:, c:c + 1], 4.0,
                                    Alu.is_equal, Alu.mult)
            for q in range(NQ):
                ps = pspool.tile([128, QCOLS], F32)
                for nb in range(QCOLS // 512):
                    for p in range(KP):
                        nc.tensor.matmul(
                            ps[:, nb * 512:(nb + 1) * 512],
                            xtb[0][:, 2 * p:2 * p + 2,
                                   PAD + c * 128:PAD + (c + 1) * 128],
                            xtb[q][:, 2 * p:2 * p + 2, nb * 512:(nb + 1) * 512],
                            start=(p == 0), stop=(p == KP - 1),
                            perf_mode=DR)
                # ACT evac: -sim (q0 to a staging tile, same4 merged below)
                acc = ({"accum_out": stage[:, C_QS + q:C_QS + q + 1]}
                       if c == CHUNKS - 1 else {})
                nc.scalar.activation(
                    q0raw[:] if q == 0 else smneg[:, q * QCOLS:(q + 1) * QCOLS],
                    ps[:], Act.Copy, bias=0.0, scale=-1.0, **acc)
            # merge class mask into quarter 0 (f16 2x TT)
            nc.vector.tensor_tensor(smneg[:, 0:QCOLS], same4[:], q0raw[:],
                                    Alu.add)
            # window rowmax -> alpha right away so ACT's scans can start
            nc.vector.tensor_reduce(stage[:, C_MAX + c:C_MAX + c + 1],
                                    smneg[:, w0:w0 + WIN], AX, Alu.max)
            nc.vector.tensor_scalar(alpha[:], stage[:, C_MAX + c:C_MAX + c + 1],
                                    -3.9, None, Alu.add)
            nc.vector.tensor_scalar(alphan[:],
                                    stage[:, C_MAX + c:C_MAX + c + 1],
                                    -1.0, 3.9, Alu.mult, Alu.add)
            nc.scalar.activation(junk_a[:, 0:RST], smneg[:, SPL:], Act.Sign,
                                 bias=alphan[:], scale=1.0,
                                 accum_out=stage[:, C_S2 + c:C_S2 + c + 1])
            nc.scalar.activation(junk_a[:, 0:RST], smneg[:, SPL:], Act.Relu,
                                 bias=alpha[:], scale=-1.0,
                                 accum_out=stage[:, C_R2 + c:C_R2 + c + 1])
            # half-row pairwise-min trees (DVE f16 2x); treeA needs only q0/q1
            t = dbuf.tile([128, 6144], F16, name="tmin")
            mm = dbuf.tile([128, 2], F32, name="mm")
            nc.vector.tensor_tensor(t[:, 0:2048], smneg[:, 0:2048],
                                    smneg[:, 2048:4096], Alu.min)
            nc.vector.tensor_tensor(t[:, 2048:3072], t[:, 0:1024],
                                    t[:, 1024:2048], Alu.min)
            nc.vector.tensor_tensor(t[:, 0:512], t[:, 2048:2560],
                                    t[:, 2560:3072], Alu.min)
            nc.vector.tensor_reduce(mm[:, 0:1], t[:, 0:512], AX, Alu.min)
            # neg side scans on DVE (need alpha and quarters 0-2)
            nc.vector.tensor_scalar(junk_d[:, 0:SPL], smneg[:, 0:SPL],
                                    alpha[:], 0.0, Alu.is_lt, Alu.add,
                                    accum_out=stage[:, C_G1 + c:C_G1 + c + 1])
            nc.vector.tensor_scalar(junk_d[:, 0:SPL], smneg[:, 0:SPL],
                                    alpha[:], 0.0, Alu.min, Alu.add,
                                    accum_out=stage[:, C_E1 + c:C_E1 + c + 1])
            # treeB over quarters 2,3
            nc.vector.tensor_tensor(t[:, 3072:5120], smneg[:, 4096:6144],
                                    smneg[:, 6144:8192], Alu.min)
            nc.vector.tensor_tensor(t[:, 5120:6144], t[:, 3072:4096],
                                    t[:, 4096:5120], Alu.min)
            nc.vector.tensor_tensor(t[:, 0:512], t[:, 5120:5632],
                                    t[:, 5632:6144], Alu.min)
            nc.vector.tensor_reduce(mm[:, 1:2], t[:, 0:512], AX, Alu.min)
            nc.vector.tensor_reduce(stage[:, C_MIN + c:C_MIN + c + 1],
                                    mm[:, 0:2], AX, Alu.min)
            nc.vector.tensor_scalar(beta[:], stage[:, C_MIN + c:C_MIN + c + 1],
                                    3.9, None, Alu.add)
            # pos side: window ops
            nc.vector.tensor_scalar(junk_w[:], smneg[:, w0:w0 + WIN],
                                    beta[:], 0.0, Alu.is_gt, Alu.add,
                                    accum_out=stage[:, C_PC + c:C_PC + c + 1])
            nc.vector.tensor_scalar(junk_w[:], smneg[:, w0:w0 + WIN],
                                    beta[:], 0.0, Alu.max, Alu.add,
                                    accum_out=stage[:, C_F + c:C_F + c + 1])

            if c == CHUNKS - 1:
                # unmined last-row stats (row n-1 = partition 127, core 7)
                nc.vector.tensor_scalar(junk_w[:], smneg[:, w0:w0 + WIN],
                                        3.0, 0.0, Alu.is_gt, Alu.add,
                                        accum_out=stage[:, C_PCALL:C_PCALL + 1])
                nc.vector.tensor_scalar(junk_w[:], smneg[:, w0:w0 + WIN],
                                        3.0, 0.0, Alu.max, Alu.add,
                                        accum_out=stage[:, C_PSALL:C_PSALL + 1])
                selfc = PAD + c * 128 + 127
                nc.vector.tensor_copy(stage[:, C_SELF:C_SELF + 1],
                                      smneg[:, selfc:selfc + 1])

        nc.sync.dma_start(st_d.ap(), stage[:])
    nc.compile()
    return nc


_NC_CACHE = None


def kernel(inputs, targets, _want_time=False, _trace=False):
    global _NC_CACHE
    x = np.asarray(inputs, dtype=np.float32)
    tgt = np.asarray(targets).astype(np.int64)
    n = N_TOT

    # class-sort rows; pin original last row to sorted position n-1 so the
    # last-row stats land at core 7 / chunk 7 / partition 127
    c_star = tgt[n - 1]
    order = np.argsort(np.where(tgt == c_star, 1 << 20, tgt), kind="stable")
    xs = x[order]
    ts_ = tgt[order].astype(np.float32)
    x8 = xs.astype(ml_dtypes.float8_e4m3fn)

    if _NC_CACHE is None:
        _NC_CACHE = build_program()
    nc = _NC_CACHE

    in_maps = []
    for m in range(N_CORES):
        shift = (m * ROWS - PAD) % n
        cols = (np.arange(n) + shift) % n
        xr = x8[cols]                       # [n, d] rotated
        xt_m = np.ascontiguousarray(
            xr.T.reshape(KP * 2, 128, n).transpose(1, 0, 2))
        tb_m = np.ascontiguousarray(np.broadcast_to(
            ts_[cols[:QCOLS]][None, :], (128, QCOLS))).astype(np.float16)
        tp_m = np.ascontiguousarray(
            ts_[m * ROWS:(m + 1) * ROWS].reshape(CHUNKS, 128).T
        ).astype(np.float32)
        in_maps.append({"xt": xt_m, "tb": tb_m, "tp": tp_m})

    res = run_bass_kernel_spmd(nc, in_maps, core_ids=list(range(N_CORES)),
                               trace=_trace)

    # ---- host finisher ----
    maxS = np.empty(n); minS = np.empty(n)
    g1 = np.empty(n); e1 = np.empty(n)
    s2 = np.empty(n); r2 = np.empty(n)
    pcnt = np.empty(n); fsum = np.empty(n)
    last = None
    for m in range(N_CORES):
        st = np.asarray(res.results[m]["stage"], dtype=np.float64)
        for c in range(CHUNKS):
            rows = slice(m * ROWS + c * 128, m * ROWS + (c + 1) * 128)
            maxS[rows] = st[:, C_MAX + c]
            minS[rows] = st[:, C_MIN + c]
            g1[rows] = st[:, C_G1 + c]
            e1[rows] = st[:, C_E1 + c]
            s2[rows] = st[:, C_S2 + c]
            r2[rows] = st[:, C_R2 + c]
            pcnt[rows] = st[:, C_PC + c]
            fsum[rows] = st[:, C_F + c]
        if m == N_CORES - 1:
            last = st

    alpha = maxS - (4.0 - MARGIN)
    beta = minS + (4.0 - MARGIN)
    g1 = np.round(g1)
    ncnt2 = np.round((RST - s2) / 2.0)
    ncnt = g1 + ncnt2
    pcnt = np.round(pcnt)
    # sum of kept smneg: DVE half via min-accum, ACT half via relu-accum
    neg_sum_smneg = (e1 - alpha * (SPL - g1)) + (alpha * ncnt2 - r2)
    neg_sum_sim = -neg_sum_smneg
    pos_sum_smneg = fsum - beta * (WIN - pcnt)
    pos_sum_sim = 4.0 * pcnt - pos_sum_smneg

    pos_loss = (pcnt - pos_sum_sim) / np.maximum(pcnt, 1.0)
    neg_loss = neg_sum_sim / np.maximum(ncnt, 1.0)
    valid = ncnt >= 1.0
    loss = np.sum(np.where(valid, pos_loss + neg_loss, 0.0)) / n
    prec = np.sum(~valid) / n

    # last-row unmined stats (partition 127 of core 7 stage)
    pc_all = float(np.round(last[127, C_PCALL]))
    sum_smneg_pos = float(last[127, C_PSALL]) - 3.0 * (WIN - pc_all)
    selfv = float(last[127, C_SELF])
    # unmined neg side from full-row quarter sums + exact class size
    cls_size = float(np.sum(tgt == c_star))
    nc_all = n - cls_size
    t_raw = float(last[127, C_QS:C_QS + 4].sum())   # sum of -sim, full row
    sum_same_smneg = (float(last[127, C_PSALL]) - 3.0 * (WIN - pc_all)
                      + (selfv if selfv <= 3.0 else 0.0))
    w_same = 4.0 * cls_size - sum_same_smneg
    neg_sim_sum = -(t_raw + w_same)
    dev_included = selfv > 3.0            # device's sim_self < 1 decision
    if INCLUDE_SELF_LAST_ROW and not dev_included:
        pc_all += 1.0; sum_smneg_pos += selfv
    elif (not INCLUDE_SELF_LAST_ROW) and dev_included:
        pc_all -= 1.0; sum_smneg_pos -= selfv
    pos_sim_sum = 4.0 * pc_all - sum_smneg_pos
    mean_pos_sim = pos_sim_sum / max(pc_all, 1.0)
    mean_neg_sim = neg_sim_sum / max(nc_all, 1.0)

    out = np.array([loss, prec, mean_pos_sim, mean_neg_sim], dtype=np.float32)
    if _want_time:
        return out, res
    return out


# revision 11
# speedup vs baseline: 1.1095x; 1.0734x over previous
"""HardMiningLoss TRN2 kernel: n=8192, d=512, 8 NeuronCores, data-parallel rows.

Encoding: smneg[i,j] = 4*same(i,j) - sim(i,j).
  negatives (diff class): smneg = -sim   in [-1, 1]
  positives (same class): smneg = 4-sim  in [ 3, 5]
Mining reductions become threshold ops on smneg:
  min_pos = 4 - rowmax(smneg);  max_neg = -rowmin(smneg)
  neg_keep: smneg < alpha, alpha = rowmax - 3.9
  pos_keep: smneg > beta,  beta  = rowmin + 3.9

Host preprocessing sorts rows by class (original last row pinned to sorted
position n-1), so each 128-row chunk's same-class columns all fall inside a
256-col window [c*128, c*128+256) after a per-core column rotation of
(core*1024 - 64). Positive-side stats (rowmax, pos cnt/sum) are window ops.

The matmul computes smneg directly: weights are -x (fp8 DoubleRow), and for
quarter 0 two extra one-hot class passes add 4*same into PSUM. Every quarter
is then evacuated by a single ACT Copy into f16 SBUF.

Engine split per chunk (128 rows x 8192 cols):
  PE   : fp8e4 DoubleRow matmuls
  ACT  : Copy evac of all quarters, Sign/Relu neg-scans on [SPL:8192]
  DVE  : window rowmax/pos ops, half-row pairwise-min trees for rowmin,
         is_lt/min neg-scans on [0:SPL]
Host finisher assembles the scalar loss from per-row linear accounting.
"""
import numpy as np
import ml_dtypes
from contextlib import ExitStack

import concourse.bass as bass
import concourse.tile as tile
from concourse import bacc, mybir
from concourse.bass_utils import run_bass_kernel_spmd

F32 = mybir.dt.float32
F16 = mybir.dt.float16
F8 = mybir.dt.float8e4
Alu = mybir.AluOpType
Act = mybir.ActivationFunctionType
AX = mybir.AxisListType.X
DR = mybir.MatmulPerfMode.DoubleRow

N_TOT, D, N_CORES = 8192, 512, 8
ROWS = N_TOT // N_CORES          # 1024 rows per core
CHUNKS = ROWS // 128             # 8 chunks of 128 rows
QCOLS = 2048                     # quarter width (half of PSUM x2 bufs)
NQ = N_TOT // QCOLS
KP = D // 256                    # 2 DoubleRow k-pair passes
PAD = 64                         # rotation pad so class windows start at c*128
WIN = 256                        # window width covering all same-class cols
WCOLS = CHUNKS * 128 + 128       # 1152: cols that can hold weights/same-class
SPL = 7040                       # DVE scans [0:SPL], ACT scans [SPL:N_TOT]
SPL_LAST = 4608                  # last chunk rebalances toward ACT (short tail)
MARGIN = 0.1
INCLUDE_SELF_LAST_ROW = True

# stage column layout
C_MAX, C_MIN, C_G1, C_E1, C_S2, C_R2, C_PC, C_F = (
    0, 8, 16, 24, 32, 40, 48, 56)
C_PCALL, C_PSALL, C_SELF, C_QS = 64, 65, 66, 67
STAGE_W = 72


def build_program():
    nc = bacc.Bacc("TRN2", target_bir_lowering=False, debug=False)
    xt_d = [nc.dram_tensor(f"xt{q}", [128, KP * 2, QCOLS], F8,
                           kind="ExternalInput") for q in range(NQ)]
    wn_d = nc.dram_tensor("wn", [128, KP * 2, WCOLS], F8, kind="ExternalInput")
    wo_d = nc.dram_tensor("wo", [128, KP * 2, WCOLS], F8, kind="ExternalInput")
    mo_d = nc.dram_tensor("mo", [128, KP * 2, WCOLS], F8, kind="ExternalInput")
    st_d = nc.dram_tensor("stage", [128, STAGE_W], F32, kind="ExternalOutput")

    with tile.TileContext(nc) as tc, ExitStack() as ctx:
        pool = ctx.enter_context(tc.tile_pool(name="p", bufs=1))
        dbuf = ctx.enter_context(tc.tile_pool(name="db", bufs=2))
        pspool = ctx.enter_context(
            tc.tile_pool(name="ps", bufs=2, space=bass.MemorySpace.PSUM))

        xtb = [pool.tile([128, KP * 2, QCOLS], F8, name=f"xtb{q}")
               for q in range(NQ)]
        wn = pool.tile([128, KP * 2, WCOLS], F8)
        wo = pool.tile([128, KP * 2, WCOLS], F8)
        mo = pool.tile([128, KP * 2, WCOLS], F8)
        stage = pool.tile([128, STAGE_W], F32)
        junk_d = pool.tile([128, SPL], F16)     # DVE scan outputs (ignored)
        junk_a = pool.tile([128, N_TOT], F8)    # ACT scan outputs (ignored)
        junk_w = pool.tile([128, WIN], F32)     # window outputs (f32: exact
                                                # beta fill values in accum)

        # DMA order matches first-chunk consumption: q1 weights+rhs first,
        # then the one-hot tensors + q0 rhs, then q2, q3
        nc.sync.dma_start(wn[:], wn_d.ap())
        nc.sync.dma_start(xtb[1][:], xt_d[1].ap())
        nc.sync.dma_start(wo[:], wo_d.ap())
        nc.sync.dma_start(mo[:], mo_d.ap())
        nc.sync.dma_start(xtb[0][:], xt_d[0].ap())
        nc.sync.dma_start(xtb[2][:], xt_d[2].ap())
        nc.sync.dma_start(xtb[3][:], xt_d[3].ap())

        for c in range(CHUNKS):
            smneg = dbuf.tile([128, N_TOT], F16, name="smneg")
            alpha = dbuf.tile([128, 1], F32, name="alpha")
            alphan = dbuf.tile([128, 1], F32, name="alphan")
            beta = dbuf.tile([128, 1], F32, name="beta")
            w0 = c * 128
            ws = slice(c * 128, c * 128 + 128)   # weight cols within wn/wo
            spl = SPL_LAST if c == CHUNKS - 1 else SPL
            rst = N_TOT - spl
            for q in (1, 0, 2, 3):
                ps = pspool.tile([128, QCOLS], F32)
                for nb in range(QCOLS // 512):
                    nbs = slice(nb * 512, (nb + 1) * 512)
                    has_oh = q == 0 and nb * 512 < WCOLS
                    for p in range(KP):
                        nc.tensor.matmul(
                            ps[:, nbs], wn[:, 2 * p:2 * p + 2, ws],
                            xtb[q][:, 2 * p:2 * p + 2, nbs],
                            start=(p == 0),
                            stop=(p == KP - 1 and not has_oh),
                            perf_mode=DR)
                    if has_oh:
                        ohw = min(512, WCOLS - nb * 512)
                        ohs = slice(nb * 512, nb * 512 + ohw)
                        for p in range(KP):
                            nc.tensor.matmul(
                                ps[:, nb * 512:nb * 512 + ohw],
                                wo[:, 2 * p:2 * p + 2, ws],
                                mo[:, 2 * p:2 * p + 2, ohs],
                                start=False, stop=(p == KP - 1),
                                perf_mode=DR)
                # ACT evac: PSUM already holds smneg (incl. 4*same on q0)
                acc = ({"accum_out": stage[:, C_QS + q:C_QS + q + 1]}
                       if c == CHUNKS - 1 else {})
                nc.scalar.activation(smneg[:, q * QCOLS:(q + 1) * QCOLS],
                                     ps[:], Act.Copy, bias=0.0, scale=1.0,
                                     **acc)
            # window rowmax -> alpha right away so ACT's scans can start
            nc.vector.tensor_reduce(stage[:, C_MAX + c:C_MAX + c + 1],
                                    smneg[:, w0:w0 + WIN], AX, Alu.max)
            nc.vector.tensor_scalar(alpha[:], stage[:, C_MAX + c:C_MAX + c + 1],
                                    -3.9, None, Alu.add)
            nc.vector.tensor_scalar(alphan[:],
                                    stage[:, C_MAX + c:C_MAX + c + 1],
                                    -1.0, 3.9, Alu.mult, Alu.add)
            nc.scalar.activation(junk_a[:, 0:rst], smneg[:, spl:], Act.Sign,
                                 bias=alphan[:], scale=1.0,
                                 accum_out=stage[:, C_S2 + c:C_S2 + c + 1])
            nc.scalar.activation(junk_a[:, 0:rst], smneg[:, spl:], Act.Relu,
                                 bias=alpha[:], scale=-1.0,
                                 accum_out=stage[:, C_R2 + c:C_R2 + c + 1])
            # half-row pairwise-min trees (DVE f16 2x); treeA needs only q0/q1
            t = dbuf.tile([128, 6144], F16, name="tmin")
            mm = dbuf.tile([128, 2], F32, name="mm")
            nc.vector.tensor_tensor(t[:, 0:2048], smneg[:, 0:2048],
                                    smneg[:, 2048:4096], Alu.min)
            nc.vector.tensor_tensor(t[:, 2048:3072], t[:, 0:1024],
                                    t[:, 1024:2048], Alu.min)
            nc.vector.tensor_tensor(t[:, 0:512], t[:, 2048:2560],
                                    t[:, 2560:3072], Alu.min)
            nc.vector.tensor_tensor(t[:, 512:768], t[:, 0:256],
                                    t[:, 256:512], Alu.min)
            nc.vector.tensor_reduce(mm[:, 0:1], t[:, 512:768], AX, Alu.min)
            # neg side scans on DVE (need alpha and quarters 0-3 per spl)
            nc.vector.tensor_scalar(junk_d[:, 0:spl], smneg[:, 0:spl],
                                    alpha[:], 0.0, Alu.is_lt, Alu.add,
                                    accum_out=stage[:, C_G1 + c:C_G1 + c + 1])
            nc.vector.tensor_scalar(junk_d[:, 0:spl], smneg[:, 0:spl],
                                    alpha[:], 0.0, Alu.min, Alu.add,
                                    accum_out=stage[:, C_E1 + c:C_E1 + c + 1])
            # treeB over quarters 2,3
            nc.vector.tensor_tensor(t[:, 3072:5120], smneg[:, 4096:6144],
                                    smneg[:, 6144:8192], Alu.min)
            nc.vector.tensor_tensor(t[:, 5120:6144], t[:, 3072:4096],
                                    t[:, 4096:5120], Alu.min)
            nc.vector.tensor_tensor(t[:, 0:512], t[:, 5120:5632],
                                    t[:, 5632:6144], Alu.min)
            nc.vector.tensor_tensor(t[:, 512:768], t[:, 0:256],
                                    t[:, 256:512], Alu.min)
            nc.vector.tensor_reduce(mm[:, 1:2], t[:, 512:768], AX, Alu.min)
            nc.vector.tensor_reduce(stage[:, C_MIN + c:C_MIN + c + 1],
                                    mm[:, 0:2], AX, Alu.min)
            nc.vector.tensor_scalar(beta[:], stage[:, C_MIN + c:C_MIN + c + 1],
                                    3.9, None, Alu.add)
            # pos side: window ops
            nc.vector.tensor_scalar(junk_w[:], smneg[:, w0:w0 + WIN],
                                    beta[:], 0.0, Alu.is_gt, Alu.add,
                                    accum_out=stage[:, C_PC + c:C_PC + c + 1])
            nc.vector.tensor_scalar(junk_w[:], smneg[:, w0:w0 + WIN],
                                    beta[:], 0.0, Alu.max, Alu.add,
                                    accum_out=stage[:, C_F + c:C_F + c + 1])

            if c == CHUNKS - 1:
                # unmined last-row stats (row n-1 = partition 127, core 7)
                nc.vector.tensor_scalar(junk_w[:], smneg[:, w0:w0 + WIN],
                                        3.0, 0.0, Alu.is_gt, Alu.add,
                                        accum_out=stage[:, C_PCALL:C_PCALL + 1])
                nc.vector.tensor_scalar(junk_w[:], smneg[:, w0:w0 + WIN],
                                        3.0, 0.0, Alu.max, Alu.add,
                                        accum_out=stage[:, C_PSALL:C_PSALL + 1])
                selfc = PAD + c * 128 + 127
                nc.vector.tensor_copy(stage[:, C_SELF:C_SELF + 1],
                                      smneg[:, selfc:selfc + 1])

        nc.sync.dma_start(st_d.ap(), stage[:])
    nc.compile()
    return nc


_NC_CACHE = None


def _pack(a):
    """[n_cols, d] fp8 -> [128, KP*2, n_cols] contraction-major tile."""
    return np.ascontiguousarray(
        a.T.reshape(KP * 2, 128, a.shape[0]).transpose(1, 0, 2))


def kernel(inputs, targets, _want_time=False, _trace=False):
    global _NC_CACHE
    x = np.asarray(inputs, dtype=np.float32)
    tgt = np.asarray(targets).astype(np.int64)
    n = N_TOT

    # class-sort rows; pin original last row to sorted position n-1 so the
    # last-row stats land at core 7 / chunk 7 / partition 127
    c_star = tgt[n - 1]
    order = np.argsort(np.where(tgt == c_star, 1 << 20, tgt), kind="stable")
    xs = x[order]
    ts_ = tgt[order]
    x8 = xs.astype(ml_dtypes.float8_e4m3fn)
    xn8 = (-xs).astype(ml_dtypes.float8_e4m3fn)
    # one-hot class encodings (value 2.0: dot of two = 4 exactly)
    eye2 = (2.0 * np.eye(D, dtype=np.float32)).astype(ml_dtypes.float8_e4m3fn)

    if _NC_CACHE is None:
        _NC_CACHE = build_program()
    nc = _NC_CACHE

    in_maps = []
    for m in range(N_CORES):
        shift = (m * ROWS - PAD) % n
        cols = (np.arange(n) + shift) % n
        tr = ts_[cols]
        im = {}
        for q in range(NQ):
            im[f"xt{q}"] = _pack(x8[cols[q * QCOLS:(q + 1) * QCOLS]])
        im["wn"] = _pack(xn8[cols[PAD:PAD + WCOLS]])
        im["wo"] = _pack(eye2[tr[PAD:PAD + WCOLS]])
        im["mo"] = _pack(eye2[tr[0:WCOLS]])
        in_maps.append(im)

    res = run_bass_kernel_spmd(nc, in_maps, core_ids=list(range(N_CORES)),
                               trace=_trace)

    # ---- host finisher ----
    maxS = np.empty(n); minS = np.empty(n)
    g1 = np.empty(n); e1 = np.empty(n)
    s2 = np.empty(n); r2 = np.empty(n)
    pcnt = np.empty(n); fsum = np.empty(n)
    spl_arr = np.empty(n)
    last = None
    for m in range(N_CORES):
        st = np.asarray(res.results[m]["stage"], dtype=np.float64)
        for c in range(CHUNKS):
            rows = slice(m * ROWS + c * 128, m * ROWS + (c + 1) * 128)
            maxS[rows] = st[:, C_MAX + c]
            minS[rows] = st[:, C_MIN + c]
            g1[rows] = st[:, C_G1 + c]
            e1[rows] = st[:, C_E1 + c]
            s2[rows] = st[:, C_S2 + c]
            r2[rows] = st[:, C_R2 + c]
            pcnt[rows] = st[:, C_PC + c]
            fsum[rows] = st[:, C_F + c]
            spl_arr[rows] = SPL_LAST if c == CHUNKS - 1 else SPL
        if m == N_CORES - 1:
            last = st

    alpha = maxS - (4.0 - MARGIN)
    beta = minS + (4.0 - MARGIN)
    g1 = np.round(g1)
    rst_arr = n - spl_arr
    ncnt2 = np.round((rst_arr - s2) / 2.0)
    ncnt = g1 + ncnt2
    pcnt = np.round(pcnt)
    # sum of kept smneg: DVE part via min-accum, ACT part via relu-accum
    neg_sum_smneg = (e1 - alpha * (spl_arr - g1)) + (alpha * ncnt2 - r2)
    neg_sum_sim = -neg_sum_smneg
    pos_sum_smneg = fsum - beta * (WIN - pcnt)
    pos_sum_sim = 4.0 * pcnt - pos_sum_smneg

    pos_loss = (pcnt - pos_sum_sim) / np.maximum(pcnt, 1.0)
    neg_loss = neg_sum_sim / np.maximum(ncnt, 1.0)
    valid = ncnt >= 1.0
    loss = np.sum(np.where(valid, pos_loss + neg_loss, 0.0)) / n
    prec = np.sum(~valid) / n

    # last-row unmined stats (partition 127 of core 7 stage)
    pc_all = float(np.round(last[127, C_PCALL]))
    sum_smneg_pos = float(last[127, C_PSALL]) - 3.0 * (WIN - pc_all)
    selfv = float(last[127, C_SELF])
    # unmined neg side from full-row quarter sums + exact class size
    cls_size = float(np.sum(tgt == c_star))
    nc_all = n - cls_size
    t_raw = float(last[127, C_QS:C_QS + 4].sum())   # sum of smneg, full row
    sum_same_smneg = (float(last[127, C_PSALL]) - 3.0 * (WIN - pc_all)
                      + (selfv if selfv <= 3.0 else 0.0))
    w_same = 4.0 * cls_size - sum_same_smneg
    # t_raw = sum(-sim) + 4*cls_size (one-hot adds on q0)
    neg_sim_sum = -((t_raw - 4.0 * cls_size) + w_same)
    dev_included = selfv > 3.0            # device's sim_self < 1 decision
    if INCLUDE_SELF_LAST_ROW and not dev_included:
        pc_all += 1.0; sum_smneg_pos += selfv
    elif (not INCLUDE_SELF_LAST_ROW) and dev_included:
        pc_all -= 1.0; sum_smneg_pos -= selfv
    pos_sim_sum = 4.0 * pc_all - sum_smneg_pos
    mean_pos_sim = pos_sim_sum / max(pc_all, 1.0)
    mean_neg_sim = neg_sim_sum / max(nc_all, 1.0)

    out = np.array([loss, prec, mean_pos_sim, mean_neg_sim], dtype=np.float32)
    if _want_time:
        return out, res
    return out
